# revision 3
# baseline (speedup 1.0000x reference)
"""CRTN middle_l query construction as a pure-DMA Bass kernel on 8 TRN2 cores.

Math (from the reference):
    query_base = concat([neighbor_mem[-1], wise_inputs], axis=0)   # (256, B, H)
    query[i, j] = query_base[i + j + 1]                            # (S, S, B, H)

For fixed i, query[i] = query_base[i+1 : i+129] is one contiguous 8 MB slab —
the whole problem is memory-bound replication: 16 MB of source fanned out to
1 GiB of output, bounded by per-core HBM write bandwidth (~358 GB/s/NC =>
~375 us floor for the 134 MB each core writes).

Sharding: data-parallel over the output axis i (S=128 -> 16 rows per core).
Core k stages query_base rows [16k+1, 16k+144) (143 rows, 9.4 MB) in SBUF,
then writes 16 contiguous 8 MB output slabs.

Layout: each 64 KB source row is split into 8 chunks of 8 KB; chunk c lives
at SBUF partition c % 128, free-offset (c // 128) * 8 KB (9 "columns",
72 KB/partition). Output row m covers chunks [8m, 8m + 1024) — with the
chunk->partition wrap this window is exactly TWO rectangles:

    A: SBUF partitions [8m, 128) x cols 0..7   -> DRAM j = 128c + p - 8m
    B: SBUF partitions [0, 8m)  x cols 1..8    -> DRAM j = 128c + p - 8m

so each 8 MB output row is TWO DMAs (one for m = 0), each reading 64 KB
contiguous per partition and writing 8 KB-contiguous DRAM blocks at 1 MB
stride (the DRAM side is the row viewed as (c p) o -> p c o). Per core:
2 staging DMAs + 31 write DMAs = 33 total (vs ~152 for the per-column
version), every one with partition start and count divisible by 8 —
measured on TRN2, partition counts not divisible by 8 fall off the HWDGE
fast path and run ~5x slower (~77 GB/s vs ~400+ GB/s).
"""

import numpy as np

import concourse.bacc as bacc
import concourse.bass as bass
import concourse.mybir as mybir
import concourse.tile as tile
from concourse.bass_utils import run_bass_kernel_spmd

# Problem shape (hardcoded; harness contract forbids reading spec.json here).
NEI_LEN = 128
S = 128
B = 16
H = 1024
N_CORES = 8
ROWS_PER_CORE = S // N_CORES          # 16 output rows (values of i) per core
IN_ROWS = ROWS_PER_CORE + S - 1       # 143 query_base rows staged per core
ROW_ELEMS = B * H                     # 16384 f32 = 64 KB per query_base row
T = 8                                 # chunks per row
CH = ROW_ELEMS // T                   # 2048 f32 = 8 KB per chunk
N_CHUNKS = T * IN_ROWS                # 1144
N_COLS = (N_CHUNKS + 127) // 128      # 9 SBUF columns
WIN = T * S                           # 1024 chunks per output row

# Timing side-channel for test harnesses (exec_time_ns when a profile ran).
LAST_EXEC_NS = None

_nc_cache = None


def _build_nc(repeats: int = 1) -> bass.Bass:
    # Bacc (not raw Bass): its compile() pass splits multi-sem waits into
    # event-semaphore chains — the walrus codegen rejects instructions with
    # more than one sync wait ("Too many sync wait commands").
    #
    # repeats > 1 unrolls the body N times (idempotent — same bytes written
    # each round); bench harnesses use the K-vs-1 slope of wall-clock exec
    # time to extract per-iteration HW time through the axon tunnel, which
    # has no NTFF profiling hook.
    nc = bacc.Bacc("TRN2", target_bir_lowering=False, debug=False)
    qb = nc.dram_tensor(
        "qb", [IN_ROWS, ROW_ELEMS], mybir.dt.float32, kind="ExternalInput"
    )
    out = nc.dram_tensor(
        "out", [ROWS_PER_CORE, WIN, CH], mybir.dt.float32, kind="ExternalOutput"
    )
    qb_flat = qb.ap().rearrange("r o -> (r o)")  # (2342912,) f32
    with tile.TileContext(nc) as tc:
        with tc.tile_pool(name="stage", bufs=min(repeats, 2)) as pool:
            for _ in range(repeats):
                buf = pool.tile([128, N_COLS * CH], mybir.dt.float32)
                # Stage chunks [0, 1024) into cols 0..7: DRAM linear order is
                # chunk-major, i.e. (c p o) with p the SBUF partition.
                nc.sync.dma_start(
                    out=buf[:, 0 : 8 * CH].rearrange("p (c o) -> p c o", c=8),
                    in_=qb_flat[0 : 1024 * CH].rearrange("(c p o) -> p c o", c=8, p=128),
                )
                # Stage chunks [1024, 1144) into col 8, partitions 0..119.
                nc.sync.dma_start(
                    out=buf[0:120, 8 * CH : 9 * CH],
                    in_=qb_flat[1024 * CH : 1144 * CH].rearrange("(p o) -> p o", p=120),
                )
                for m in range(ROWS_PER_CORE):
                    # DRAM row m as (p', c, o): element (p', c, o) sits at
                    # chunk index j = 128c + p' of the row.
                    rowv = out.ap()[m].rearrange("(c p) o -> p c o", c=8)
                    p0 = T * m
                    # A: window chunks with partition >= p0, cols 0..7.
                    nc.sync.dma_start(
                        out=rowv[0 : 128 - p0],
                        in_=buf[p0:128, 0 : 8 * CH],
                    )
                    if m:
                        # B: window chunks with partition < p0, cols 1..8.
                        nc.sync.dma_start(
                            out=rowv[128 - p0 : 128],
                            in_=buf[0:p0, CH : 9 * CH],
                        )
    nc.compile()
    return nc


def kernel(neighbor_mem: np.ndarray, wise_inputs: np.ndarray) -> np.ndarray:
    global _nc_cache, LAST_EXEC_NS
    assert neighbor_mem.shape == (13, NEI_LEN, B, H), neighbor_mem.shape
    assert wise_inputs.shape == (S, B, H), wise_inputs.shape

    qb_full = np.concatenate(
        [
            np.asarray(neighbor_mem[-1], dtype=np.float32).reshape(NEI_LEN, ROW_ELEMS),
            np.asarray(wise_inputs, dtype=np.float32).reshape(S, ROW_ELEMS),
        ],
        axis=0,
    )  # (256, 16384)

    in_maps = [
        {"qb": qb_full[ROWS_PER_CORE * k + 1 : ROWS_PER_CORE * k + 1 + IN_ROWS]}
        for k in range(N_CORES)
    ]

    if _nc_cache is None:
        _nc_cache = _build_nc()

    res = run_bass_kernel_spmd(_nc_cache, in_maps, core_ids=list(range(N_CORES)))
    LAST_EXEC_NS = res.exec_time_ns

    # out[m, k, :] with k = 8j + t is exactly row-major (S, B, H) per m.
    out = np.concatenate(
        [r["out"].reshape(ROWS_PER_CORE, S, B, H) for r in res.results], axis=0
    )
    return out


# revision 4
# speedup vs baseline: 1.5305x; 1.5305x over previous
"""CRTN middle_l query construction as a pure-DMA Bass kernel on 8 TRN2 cores.

Padded mega-column design — 11 DMAs per core.

Output rows are padded 1024 -> 1144 chunks; row m of the padded tensor holds
chunk j at flat elem m*2342912 + j*2048 (j may exceed 1024: pad garbage).
Mega-col DMA c writes, for ALL m in one 3-dim-AP DMA, SBUF col c (chunk
128c+p) to position j = 128c + p - 8m, i.e. flat m*(2342912-16384) +
c*262144 + p*2048. Out-of-row positions (j<0 from col 0 when p<8m, j>=1024
from col 8) land in padding -> discarded by the host. Every in-row position
is written exactly once => no WAW hazards, no ordering constraints.

DMAs: S1 (cols 0-7, 8MB), S2 (col 8), then 9 mega-cols (~16 MB each,
uniform across all 128 partitions -> full 16-SDMA-engine utilization)."""

import numpy as np

import concourse.bacc as bacc
import concourse.bass as bass
import concourse.mybir as mybir
import concourse.tile as tile
from concourse.bass_utils import run_bass_kernel_spmd

NEI_LEN = 128
S = 128
B = 16
H = 1024
N_CORES = 8
ROWS_PER_CORE = S // N_CORES          # 16
IN_ROWS = ROWS_PER_CORE + S - 1       # 143
ROW_ELEMS = B * H                     # 16384 f32
T = 8
CH = ROW_ELEMS // T                   # 2048 f32 = 8 KB
N_CHUNKS = T * IN_ROWS                # 1144
WIN = T * S                           # 1024 chunks per output row
ROWP = 1144                           # padded row length in chunks
RSTRIDE = ROWP * CH                   # 2342912 elems per padded row
MSTRIDE = RSTRIDE - T * CH            # 2326528: flat stride of the m-dim
OUT_ELEMS = CH * CH + 15 * MSTRIDE + 15 * CH * CH  # see below

# col-8 slice needs offset (MSTRIDE + 1024*CH) + 15*MSTRIDE; col-7 needs
# 7*128*CH + 16*MSTRIDE. Take the max, rounded up: both fit in 39321600.
OUT_ELEMS = 39321600

LAST_EXEC_NS = None
_nc_cache = None


def _build_nc(repeats: int = 1) -> bass.Bass:
    nc = bacc.Bacc("TRN2", target_bir_lowering=False, debug=False)
    qb = nc.dram_tensor(
        "qb", [IN_ROWS, ROW_ELEMS], mybir.dt.float32, kind="ExternalInput"
    )
    out = nc.dram_tensor("out", [OUT_ELEMS], mybir.dt.float32, kind="ExternalOutput")
    out_flat = out.ap()
    qb_flat = qb.ap().rearrange("r o -> (r o)")
    with tile.TileContext(nc) as tc:
        with tc.tile_pool(name="stage", bufs=min(repeats, 2)) as pool:
            for _ in range(repeats):
                buf = pool.tile([128, 9 * CH], mybir.dt.float32)
                nc.sync.dma_start(
                    out=buf[:, 0 : 8 * CH].rearrange("p (c o) -> p c o", c=8),
                    in_=qb_flat[0 : 1024 * CH].rearrange("(c p o) -> p c o", c=8, p=128),
                )
                nc.scalar.dma_start(
                    out=buf[0:120, 8 * CH : 9 * CH],
                    in_=qb_flat[1024 * CH : 1144 * CH].rearrange("(p o) -> p o", p=120),
                )
                for c in range(8):
                    # all 16 rows' copies of col c: flat = c*128*CH + m*MSTRIDE
                    # + p*CH + o, dims (p, m, o)
                    dram = (
                        out_flat[c * 128 * CH : c * 128 * CH + 16 * MSTRIDE]
                        .rearrange("(m x) -> m x", m=16)[:, 0 : 128 * CH]
                        .rearrange("m (p o) -> p m o", p=128)
                    )
                    sb = (
                        buf[:, c * CH : (c + 1) * CH]
                        .unsqueeze(1)
                        .broadcast_to([128, 16, CH])
                    )
                    eng = nc.sync if c % 2 == 0 else nc.scalar
                    eng.dma_start(out=dram, in_=sb)
                # col 8: m in [1,16) (m=0 writes only pad), 120 partitions
                base8 = MSTRIDE + 1024 * CH
                dram8 = (
                    out_flat[base8 : base8 + 15 * MSTRIDE]
                    .rearrange("(m x) -> m x", m=15)[:, 0 : 120 * CH]
                    .rearrange("m (p o) -> p m o", p=120)
                )
                sb8 = (
                    buf[0:120, 8 * CH : 9 * CH]
                    .unsqueeze(1)
                    .broadcast_to([120, 15, CH])
                )
                nc.sync.dma_start(out=dram8, in_=sb8)
    nc.compile()
    return nc


def kernel(neighbor_mem: np.ndarray, wise_inputs: np.ndarray) -> np.ndarray:
    global _nc_cache, LAST_EXEC_NS
    assert neighbor_mem.shape == (13, NEI_LEN, B, H), neighbor_mem.shape
    assert wise_inputs.shape == (S, B, H), wise_inputs.shape

    qb_full = np.concatenate(
        [
            np.asarray(neighbor_mem[-1], dtype=np.float32).reshape(NEI_LEN, ROW_ELEMS),
            np.asarray(wise_inputs, dtype=np.float32).reshape(S, ROW_ELEMS),
        ],
        axis=0,
    )  # (256, 16384)

    in_maps = [
        {"qb": qb_full[ROWS_PER_CORE * k + 1 : ROWS_PER_CORE * k + 1 + IN_ROWS]}
        for k in range(N_CORES)
    ]

    if _nc_cache is None:
        _nc_cache = _build_nc()

    res = run_bass_kernel_spmd(_nc_cache, in_maps, core_ids=list(range(N_CORES)))
    LAST_EXEC_NS = res.exec_time_ns

    parts = []
    for r in res.results:
        o = r["out"][: 16 * RSTRIDE].reshape(ROWS_PER_CORE, ROWP, CH)
        parts.append(o[:, :WIN, :].reshape(ROWS_PER_CORE, S, B, H))
    return np.concatenate(parts, axis=0)
